# revision 9
# baseline (speedup 1.0000x reference)
"""LoCon1d (position-specific conv1d) Trainium2 kernel.

out[b,o,s] = sum_{c,k} xpad[b,c,s+k] * w[o,c,s,k] + bias[o,s]
shapes: x (16,64,1024) f32, w (64,64,1024,3) f32, bias (64,1024) f32.

Sharding: sequence-parallel over 8 cores, 128 positions each.

Per-core mapping (shifted-stationary, diagonal extraction):
  Positions split into half-blocks (j, 64+j), j in 0..63, packed
  block-diagonally into the 128-partition contraction dim:
  partitions 0:64 = Cin for position j, 64:128 = Cin for position
  64+j; batch columns 0:16 <-> j, 16:32 <-> 64+j (zeros elsewhere,
  baked in on host).

  Windows of 2 consecutive pairs (p in {0,1}, j = 2w+p). Per window,
  3 accumulating matmuls (one per tap kk) share one PSUM [64,128]:
    lhsT (stationary) = xr[:, 2w+kk : 2w+kk+2, :]  -> [128, 2*32]
    rhs  (moving)     = wr[:, w, kk, :, :]         -> [128, 2*64]
  Slot [32a+m, 64p+o] accumulates sum_kk x[2w+kk+a] * w[pair p, kk];
  the diagonal a==p holds the true output, off-diagonal is garbage
  that the host discards. No on-device tap-sum or bias needed (bias
  is added during host-side assembly).

  The kernel is HBM-bound on the weight (3.1 MiB f16 per core), so
  DMA is scheduled for earliest-first completion: x in 4 slabs on
  sync, weight in progressively-sized chunks (1,1,2,4,8,8,8 windows)
  alternating scalar/gpsimd so window 0 lands ~0.7us after issue and
  the PE streams right behind the DMA. PSUM->f16 copies run on
  vector (plus scalar for the tail); out DMAs (f16) sit on sync.
"""

import numpy as np

import concourse.bass as bass
import concourse.mybir as mybir
import concourse.tile as tile
from concourse import bacc, bass_utils

N_CORES = 8
B, CIN, COUT, S, K = 16, 64, 64, 1024, 3
SC = S // N_CORES          # positions per core (128)
H = SC // 2                # half-block (64)
W = H // 2                 # windows per core (32)
TW = H + K - 1             # x window length per half-block (66)
XG = 4                     # x slabs
XWIN = 8                   # windows per x slab
XT = 2 * XWIN + 2          # t-slices per x slab (18)
WCHUNKS = [1, 1, 2, 4, 8, 8, 8]   # weight chunk sizes in windows
OGROUPS = [8, 8, 8, 4, 4]         # out DMA group sizes in windows

_DT = {"f32": mybir.dt.float32, "bf16": mybir.dt.bfloat16,
       "f16": mybir.dt.float16}

DTYPE = "f16"


def _np_dt(dt):
    if dt == "bf16":
        import ml_dtypes
        return ml_dtypes.bfloat16
    if dt == "f16":
        return np.float16
    return np.float32


def build_bass(dtype=DTYPE):
    dt = _DT[dtype]
    nc = bacc.Bacc("TRN2", target_bir_lowering=False, debug=False,
                   num_devices=N_CORES)
    xr = nc.dram_tensor("xr", [128, TW, 32], dt, kind="ExternalInput")
    wr = nc.dram_tensor("wr", [128, W, K, 128], dt, kind="ExternalInput")
    out = nc.dram_tensor("out", [64, W, 128], dt, kind="ExternalOutput")

    with tile.TileContext(nc) as tc:
        import contextlib
        n_sizes = sorted(set(WCHUNKS))
        with (
            tc.tile_pool(name="xpool", bufs=XG) as xpool,
            tc.tile_pool(name="opool", bufs=len(OGROUPS)) as opool,
            tc.tile_pool(name="psum", bufs=8, space="PSUM") as pspool,
            contextlib.ExitStack() as wstack,
        ):
            wpool_by_size = {
                n: wstack.enter_context(
                    tc.tile_pool(name=f"w{n}", bufs=WCHUNKS.count(n)))
                for n in n_sizes}
            # x slabs on sync, earliest first; slab g covers windows
            # [8g, 8g+8) via t-slices [16g, 16g+18)
            x_t = []
            for g in range(XG):
                xt = xpool.tile([128, XT, 32], dt, tag="xt")
                nc.sync.dma_start(out=xt[:, :, :],
                                  in_=xr.ap()[:, 16 * g:16 * g + XT, :])
                x_t.append(xt)
            # weight chunks, progressive sizes, alternating scalar/gpsimd
            w_engs = [nc.scalar, nc.gpsimd]
            w_tiles = []       # per window: (tile, local w index)
            w0 = 0
            for ci, nw in enumerate(WCHUNKS):
                wt = wpool_by_size[nw].tile([128, nw, K, 128], dt,
                                            tag=f"wt{nw}")
                w_engs[ci % 2].dma_start(
                    out=wt[:, :, :, :],
                    in_=wr.ap()[:, w0:w0 + nw, :, :])
                for i in range(nw):
                    w_tiles.append((wt, i))
                w0 += nw

            w = 0
            for gi, gn in enumerate(OGROUPS):
                o_t = opool.tile([64, gn, 128], dt, tag="ot")
                og0 = w
                for wi in range(gn):
                    xt = x_t[w // XWIN]
                    t0 = 2 * w - 16 * (w // XWIN)
                    wt, li = w_tiles[w]
                    ps = pspool.tile([64, 128], mybir.dt.float32, tag="ps")
                    for kk in range(K):
                        nc.tensor.matmul(
                            ps[:, :],
                            lhsT=xt[:, t0 + kk:t0 + kk + 2, :],
                            rhs=wt[:, li, kk, :],
                            start=(kk == 0),
                            stop=(kk == K - 1),
                        )
                    # early copies on vector (no DMA duty); tail copies on
                    # scalar once its weight DMA issues have drained
                    if w < 24:
                        nc.vector.tensor_copy(out=o_t[:, wi, :], in_=ps[:, :])
                    else:
                        nc.scalar.copy(out=o_t[:, wi, :], in_=ps[:, :])
                    w += 1
                nc.sync.dma_start(out=out.ap()[:, og0:og0 + gn, :],
                                  in_=o_t[:, :, :])
    nc.compile()
    return nc


def prep_inputs(input, weight, bias, dtype=DTYPE):
    """Host-side shard + relayout. Returns list of per-core input dicts."""
    npdt = _np_dt(dtype)
    xpad = np.pad(np.asarray(input, np.float32), ((0, 0), (0, 0), (1, 1)))
    w = np.asarray(weight, np.float32)
    in_maps = []
    for i in range(N_CORES):
        s0 = i * SC
        # x: [p, t, b_ext] block-diagonal
        xa = xpad[:, :, s0:s0 + TW]             # (B, CIN, TW)
        xb = xpad[:, :, s0 + H:s0 + H + TW]
        xr = np.zeros((128, TW, 32), np.float32)
        xr[:64, :, :16] = xa.transpose(1, 2, 0)
        xr[64:, :, 16:] = xb.transpose(1, 2, 0)
        # w: [p, w, kk, (p2, o)]
        ws = w[:, :, s0:s0 + SC, :]             # (COUT, CIN, SC, K)
        wa = ws[:, :, :H, :].reshape(COUT, CIN, W, 2, K)
        wb = ws[:, :, H:, :].reshape(COUT, CIN, W, 2, K)
        wr = np.empty((128, W, K, 2, COUT), np.float32)
        wr[:64] = wa.transpose(1, 2, 4, 3, 0)   # (c, w, kk, p2, o)
        wr[64:] = wb.transpose(1, 2, 4, 3, 0)
        in_maps.append({
            "xr": np.ascontiguousarray(xr.astype(npdt)),
            "wr": np.ascontiguousarray(
                wr.reshape(128, W, K, 128).astype(npdt)),
        })
    return in_maps


def assemble_output(results, bias):
    full = np.empty((B, COUT, S), np.float32)
    for i, r in enumerate(results):
        s0 = i * SC
        oc = np.asarray(r["out"], np.float32)    # (64, W, 128)
        for p in range(2):
            blk = oc[32 * p:32 * p + 32, :, 64 * p:64 * p + 64]  # (m, w, o)
            # half A: positions s0 + 2w + p ; half B: s0 + 64 + 2w + p
            full[:, :, s0 + p:s0 + H:2] = blk[:16].transpose(0, 2, 1)
            full[:, :, s0 + H + p:s0 + SC:2] = blk[16:].transpose(0, 2, 1)
    full += np.asarray(bias, np.float32)[None, :, :]
    return full


_CACHED = {}


def run(inputs, dtype=DTYPE, trace=False):
    if dtype not in _CACHED:
        _CACHED[dtype] = build_bass(dtype)
    nc = _CACHED[dtype]
    in_maps = prep_inputs(inputs["input"], inputs["weight"], inputs["bias"],
                          dtype)
    res = bass_utils.run_bass_kernel_spmd(
        nc, in_maps, core_ids=list(range(N_CORES)), trace=trace)
    return assemble_output(res.results, inputs["bias"]), res


def kernel(input, weight, bias):
    out, _ = run({"input": input, "weight": weight, "bias": bias},
                 trace=False)
    return out


# revision 16
# speedup vs baseline: 1.0314x; 1.0314x over previous
"""LoCon1d (position-specific conv1d) Trainium2 kernel.

out[b,o,s] = sum_{c,k} xpad[b,c,s+k] * w[o,c,s,k] + bias[o,s]
shapes: x (16,64,1024) f32, w (64,64,1024,3) f32, bias (64,1024) f32.

Sharding: sequence-parallel over 8 cores, 128 positions each.

Per-core mapping (shifted-stationary, diagonal extraction):
  Positions split into half-blocks (j, 64+j), j in 0..63, packed
  block-diagonally into the 128-partition contraction dim:
  partitions 0:64 = Cin for position j, 64:128 = Cin for position
  64+j; batch columns 0:16 <-> j, 16:32 <-> 64+j (zero blocks
  memset on device, compact halves DMAed in).

  Windows of 2 consecutive pairs (p in {0,1}, j = 2w+p). Per window,
  3 accumulating matmuls (one per tap kk) share one PSUM [64,128]:
    lhsT (stationary) = xt[:, :, t0+kk : t0+kk+2]  -> [128, 32*2]
    rhs  (moving)     = wr[:, w, kk, :]            -> [128, 2*64]
  PSUM row = 2*m + a (m = batch-ext column, a = stationary t-slot);
  slot a==p holds the true output for pair p, a!=p is garbage. The
  out DMA reads only the valid rows (a==p interleave) per p-block,
  so no garbage reaches DRAM. Bias is added during host assembly.

  The kernel is HBM-bound on the weight (3.1 MiB f16 per core):
  weight chunks are progressively sized (1,1,2,4,8,8 windows) on the
  two HWDGE engines (scalar, sync) so window 0 lands early and the
  PE streams right behind the DMA; gpsimd (slow non-HWDGE DMA path)
  only carries the final chunk, issued first, needed last.
"""

import numpy as np

import concourse.bass as bass
import concourse.mybir as mybir
import concourse.tile as tile
from concourse import bacc, bass_utils

N_CORES = 8
B, CIN, COUT, S, K = 16, 64, 64, 1024, 3
SC = S // N_CORES          # positions per core (128)
H = SC // 2                # half-block (64)
W = H // 2                 # windows per core (32)
TW = H + K - 1             # x window length per half-block (66)
OGROUPS = [16, 12, 4]      # out DMA group sizes in windows

_DT = {"f32": mybir.dt.float32, "bf16": mybir.dt.bfloat16,
       "f16": mybir.dt.float16}

DTYPE = "f16"


def _np_dt(dt):
    if dt == "bf16":
        import ml_dtypes
        return ml_dtypes.bfloat16
    if dt == "f16":
        return np.float16
    return np.float32


def build_bass(dtype=DTYPE):
    dt = _DT[dtype]
    nc = bacc.Bacc("TRN2", target_bir_lowering=False, debug=False,
                   num_devices=N_CORES)
    xr = nc.dram_tensor("xr", [128, TW, 32], dt, kind="ExternalInput")
    wr = nc.dram_tensor("wr", [128, W, K, 128], dt, kind="ExternalInput")
    # rows 0:32 = pair p=0 (valid psum rows 2m+0), 32:64 = pair p=1
    out = nc.dram_tensor("out", [64, W, 64], dt, kind="ExternalOutput")

    # (window start, n windows, engine index) ; engines resolved below
    wchunks = [(0, 1, 0), (1, 1, 0), (2, 2, 0), (4, 4, 0), (8, 8, 0),
               (16, 8, 1), (24, 4, 1), (28, 4, 2)]

    import contextlib
    with tile.TileContext(nc) as tc:
        n_sizes = sorted({c[1] for c in wchunks})
        with (
            tc.tile_pool(name="xpool", bufs=1) as xpool,
            tc.tile_pool(name="opool", bufs=len(OGROUPS)) as opool,
            tc.tile_pool(name="psum", bufs=8, space="PSUM") as pspool,
            contextlib.ExitStack() as wstack,
        ):
            wpools = {
                n: wstack.enter_context(tc.tile_pool(
                    name=f"w{n}",
                    bufs=sum(1 for c in wchunks if c[1] == n)))
                for n in n_sizes}

            w_engs = [nc.scalar, nc.sync, nc.gpsimd]
            xt = xpool.tile([128, TW, 32], dt)
            nc.sync.dma_start(out=xt[:, :, :], in_=xr.ap())

            w_tiles = [None] * W     # per window: (tile, local index)
            for w0, nw, ei in wchunks:
                wt = wpools[nw].tile([128, nw, K, 128], dt, tag=f"wt{nw}")
                w_engs[ei].dma_start(
                    out=wt[:, :, :, :],
                    in_=wr.ap()[:, w0:w0 + nw, :, :])
                for i in range(nw):
                    w_tiles[w0 + i] = (wt, i)

            w = 0
            for gi, gn in enumerate(OGROUPS):
                o_t = opool.tile([64, gn, 128], dt, tag="ot")
                og0 = w
                for wi in range(gn):
                    wt, li = w_tiles[w]
                    ps = pspool.tile([64, 128], mybir.dt.float32, tag="ps")
                    for kk in range(K):
                        nc.tensor.matmul(
                            ps[:, :],
                            lhsT=xt[:, 2 * w + kk:2 * w + kk + 2, :],
                            rhs=wt[:, li, kk, :],
                            start=(kk == 0),
                            stop=(kk == K - 1),
                        )
                    # early copies on vector (no DMA duty); tail copies on
                    # scalar once its weight DMA issues have drained
                    if w < 24:
                        nc.vector.tensor_copy(out=o_t[:, wi, :], in_=ps[:, :])
                    else:
                        nc.scalar.copy(out=o_t[:, wi, :], in_=ps[:, :])
                    w += 1
                # valid rows for pair p: psum/copy rows 32p+m, cols 64p+o
                nc.sync.dma_start(
                    out=out.ap()[0:32, og0:og0 + gn, :],
                    in_=o_t[0:32, :, 0:64])
                nc.scalar.dma_start(
                    out=out.ap()[32:64, og0:og0 + gn, :],
                    in_=o_t[32:64, :, 64:128])
    nc.compile()
    return nc


def prep_inputs(input, weight, bias, dtype=DTYPE):
    """Host-side shard + relayout. Returns list of per-core input dicts."""
    npdt = _np_dt(dtype)
    xpad = np.pad(np.asarray(input, np.float32), ((0, 0), (0, 0), (1, 1)))
    w = np.asarray(weight, np.float32)
    in_maps = []
    for i in range(N_CORES):
        s0 = i * SC
        # x: [p, t, b_ext] block-diagonal
        xa = xpad[:, :, s0:s0 + TW]             # (B, CIN, TW)
        xb = xpad[:, :, s0 + H:s0 + H + TW]
        xrc = np.zeros((128, TW, 32), np.float32)
        xrc[:64, :, :16] = xa.transpose(1, 2, 0)
        xrc[64:, :, 16:] = xb.transpose(1, 2, 0)
        # w: [p, w, kk, (p2, o)]
        ws = w[:, :, s0:s0 + SC, :]             # (COUT, CIN, SC, K)
        wa = ws[:, :, :H, :].reshape(COUT, CIN, W, 2, K)
        wb = ws[:, :, H:, :].reshape(COUT, CIN, W, 2, K)
        wr = np.empty((128, W, K, 2, COUT), np.float32)
        wr[:64] = wa.transpose(1, 2, 4, 3, 0)   # (c, w, kk, p2, o)
        wr[64:] = wb.transpose(1, 2, 4, 3, 0)
        in_maps.append({
            "xr": np.ascontiguousarray(xrc.astype(npdt)),
            "wr": np.ascontiguousarray(
                wr.reshape(128, W, K, 128).astype(npdt)),
        })
    return in_maps


def assemble_output(results, bias):
    full = np.empty((B, COUT, S), np.float32)
    for i, r in enumerate(results):
        s0 = i * SC
        oc = np.asarray(r["out"], np.float32)    # (64, W, 64)
        for p in range(2):
            blk = oc[32 * p:32 * p + 32, :, :]   # (m, w, o)
            # half A: positions s0 + 2w + p ; half B: s0 + 64 + 2w + p
            full[:, :, s0 + p:s0 + H:2] = blk[:16].transpose(0, 2, 1)
            full[:, :, s0 + H + p:s0 + SC:2] = blk[16:].transpose(0, 2, 1)
    full += np.asarray(bias, np.float32)[None, :, :]
    return full


_CACHED = {}


def run(inputs, dtype=DTYPE, trace=False):
    if dtype not in _CACHED:
        _CACHED[dtype] = build_bass(dtype)
    nc = _CACHED[dtype]
    in_maps = prep_inputs(inputs["input"], inputs["weight"], inputs["bias"],
                          dtype)
    res = bass_utils.run_bass_kernel_spmd(
        nc, in_maps, core_ids=list(range(N_CORES)), trace=trace)
    return assemble_output(res.results, inputs["bias"]), res


def kernel(input, weight, bias):
    out, _ = run({"input": input, "weight": weight, "bias": bias},
                 trace=False)
    return out


# revision 20
# speedup vs baseline: 1.0381x; 1.0065x over previous
"""LoCon1d (position-specific conv1d) Trainium2 kernel.

out[b,o,s] = sum_{c,k} xpad[b,c,s+k] * w[o,c,s,k] + bias[o,s]
shapes: x (16,64,1024) f32, w (64,64,1024,3) f32, bias (64,1024) f32.

Sharding: sequence-parallel over 8 cores, 128 positions each.

Per-core mapping (shifted-stationary, diagonal extraction):
  Positions split into half-blocks (j, 64+j), j in 0..63, packed
  block-diagonally into the 128-partition contraction dim:
  partitions 0:64 = Cin for position j, 64:128 = Cin for position
  64+j; batch columns 0:16 <-> j, 16:32 <-> 64+j (zero blocks
  memset on device, compact halves DMAed in).

  Windows of 2 consecutive pairs (p in {0,1}, j = 2w+p). Per window,
  3 accumulating matmuls (one per tap kk) share one PSUM [64,128]:
    lhsT (stationary) = xt[:, :, t0+kk : t0+kk+2]  -> [128, 32*2]
    rhs  (moving)     = wr[:, w, kk, :]            -> [128, 2*64]
  PSUM row = 2*m + a (m = batch-ext column, a = stationary t-slot);
  slot a==p holds the true output for pair p, a!=p is garbage. The
  out DMA reads only the valid rows (a==p interleave) per p-block,
  so no garbage reaches DRAM. Bias is added during host assembly.

  The kernel is HBM-bound on the weight (3.1 MiB f16 per core):
  weight chunks are progressively sized (1,1,2,4,8,8 windows) on the
  two HWDGE engines (scalar, sync) so window 0 lands early and the
  PE streams right behind the DMA; gpsimd (slow non-HWDGE DMA path)
  only carries the final chunk, issued first, needed last.
"""

import numpy as np

import concourse.bass as bass
import concourse.mybir as mybir
import concourse.tile as tile
from concourse import bacc, bass_utils

N_CORES = 8
B, CIN, COUT, S, K = 16, 64, 64, 1024, 3
SC = S // N_CORES          # positions per core (128)
H = SC // 2                # half-block (64)
W = H // 2                 # windows per core (32)
TW = H + K - 1             # x window length per half-block (66)
OGROUPS = [8, 8, 8, 8]     # out DMA group sizes in windows

_DT = {"f32": mybir.dt.float32, "bf16": mybir.dt.bfloat16,
       "f16": mybir.dt.float16}

DTYPE = "f16"


def _np_dt(dt):
    if dt == "bf16":
        import ml_dtypes
        return ml_dtypes.bfloat16
    if dt == "f16":
        return np.float16
    return np.float32


def build_bass(dtype=DTYPE):
    dt = _DT[dtype]
    nc = bacc.Bacc("TRN2", target_bir_lowering=False, debug=False,
                   num_devices=N_CORES)
    xr = nc.dram_tensor("xr", [128, TW, 32], dt, kind="ExternalInput")
    wr = nc.dram_tensor("wr", [128, W, K, 128], dt, kind="ExternalInput")
    out = nc.dram_tensor("out", [64, W, 128], dt, kind="ExternalOutput")

    # (window start, n windows, engine index): scalar streams the head
    # progressively, sync takes the middle after xr, gpsimd (slower
    # independent DMA path, ~3us extra latency) takes the tail early.
    wchunks = [(0, 1, 0), (1, 1, 0), (2, 2, 0), (4, 4, 0), (8, 8, 0),
               (16, 4, 1), (20, 4, 1), (24, 2, 1), (26, 6, 2)]

    import contextlib
    with tile.TileContext(nc) as tc:
        n_sizes = sorted({c[1] for c in wchunks})
        with (
            tc.tile_pool(name="xpool", bufs=1) as xpool,
            tc.tile_pool(name="opool", bufs=len(OGROUPS)) as opool,
            tc.tile_pool(name="psum", bufs=8, space="PSUM") as pspool,
            contextlib.ExitStack() as wstack,
        ):
            wpools = {
                n: wstack.enter_context(tc.tile_pool(
                    name=f"w{n}",
                    bufs=sum(1 for c in wchunks if c[1] == n)))
                for n in n_sizes}

            w_engs = [nc.scalar, nc.sync, nc.gpsimd]
            xt = xpool.tile([128, TW, 32], dt)
            nc.sync.dma_start(out=xt[:, :, :], in_=xr.ap())

            w_tiles = [None] * W     # per window: (tile, local index)
            for w0, nw, ei in wchunks:
                wt = wpools[nw].tile([128, nw, K, 128], dt, tag=f"wt{nw}")
                w_engs[ei].dma_start(
                    out=wt[:, :, :, :],
                    in_=wr.ap()[:, w0:w0 + nw, :, :])
                for i in range(nw):
                    w_tiles[w0 + i] = (wt, i)

            w = 0
            for gi, gn in enumerate(OGROUPS):
                o_t = opool.tile([64, gn, 128], dt, tag="ot")
                og0 = w
                for wi in range(gn):
                    wt, li = w_tiles[w]
                    ps = pspool.tile([64, 128], mybir.dt.float32, tag="ps")
                    for kk in range(K):
                        nc.tensor.matmul(
                            ps[:, :],
                            lhsT=xt[:, 2 * w + kk:2 * w + kk + 2, :],
                            rhs=wt[:, li, kk, :],
                            start=(kk == 0),
                            stop=(kk == K - 1),
                        )
                    # early copies on vector (no DMA duty); tail copies on
                    # scalar once its weight DMA issues have drained
                    if w < 24:
                        nc.vector.tensor_copy(out=o_t[:, wi, :], in_=ps[:, :])
                    else:
                        nc.scalar.copy(out=o_t[:, wi, :], in_=ps[:, :])
                    w += 1
                nc.sync.dma_start(
                    out=out.ap()[:, og0:og0 + gn, :],
                    in_=o_t[:, :, :])
    nc.compile()
    return nc


def prep_inputs(input, weight, bias, dtype=DTYPE):
    """Host-side shard + relayout. Returns list of per-core input dicts."""
    npdt = _np_dt(dtype)
    xpad = np.pad(np.asarray(input, np.float32), ((0, 0), (0, 0), (1, 1)))
    w = np.asarray(weight, np.float32)
    in_maps = []
    for i in range(N_CORES):
        s0 = i * SC
        # x: [p, t, b_ext] block-diagonal
        xa = xpad[:, :, s0:s0 + TW]             # (B, CIN, TW)
        xb = xpad[:, :, s0 + H:s0 + H + TW]
        xrc = np.zeros((128, TW, 32), np.float32)
        xrc[:64, :, :16] = xa.transpose(1, 2, 0)
        xrc[64:, :, 16:] = xb.transpose(1, 2, 0)
        # w: [p, w, kk, (p2, o)]
        ws = w[:, :, s0:s0 + SC, :]             # (COUT, CIN, SC, K)
        wa = ws[:, :, :H, :].reshape(COUT, CIN, W, 2, K)
        wb = ws[:, :, H:, :].reshape(COUT, CIN, W, 2, K)
        wr = np.empty((128, W, K, 2, COUT), np.float32)
        wr[:64] = wa.transpose(1, 2, 4, 3, 0)   # (c, w, kk, p2, o)
        wr[64:] = wb.transpose(1, 2, 4, 3, 0)
        in_maps.append({
            "xr": np.ascontiguousarray(xrc.astype(npdt)),
            "wr": np.ascontiguousarray(
                wr.reshape(128, W, K, 128).astype(npdt)),
        })
    return in_maps


def assemble_output(results, bias):
    full = np.empty((B, COUT, S), np.float32)
    for i, r in enumerate(results):
        s0 = i * SC
        oc = np.asarray(r["out"], np.float32)    # (64, W, 128)
        for p in range(2):
            blk = oc[32 * p:32 * p + 32, :, 64 * p:64 * p + 64]  # (m, w, o)
            # half A: positions s0 + 2w + p ; half B: s0 + 64 + 2w + p
            full[:, :, s0 + p:s0 + H:2] = blk[:16].transpose(0, 2, 1)
            full[:, :, s0 + H + p:s0 + SC:2] = blk[16:].transpose(0, 2, 1)
    full += np.asarray(bias, np.float32)[None, :, :]
    return full


_CACHED = {}


def run(inputs, dtype=DTYPE, trace=False):
    if dtype not in _CACHED:
        _CACHED[dtype] = build_bass(dtype)
    nc = _CACHED[dtype]
    in_maps = prep_inputs(inputs["input"], inputs["weight"], inputs["bias"],
                          dtype)
    res = bass_utils.run_bass_kernel_spmd(
        nc, in_maps, core_ids=list(range(N_CORES)), trace=trace)
    return assemble_output(res.results, inputs["bias"]), res


def kernel(input, weight, bias):
    out, _ = run({"input": input, "weight": weight, "bias": bias},
                 trace=False)
    return out
